# revision 24
# baseline (speedup 1.0000x reference)
"""Trainium2 Bass kernel for nn_Attention_77146202570808.

Dual-stream (protein/molecule) multi-head attention block:
  q/k/v projections for both streams, 4 attention passes (pp, mm, pm, mp),
  a Linear over the *sequence* axis (P+M -> P / M), and output projections.

Sharding: data-parallel over batch. B=8 batches, 8 NeuronCores, one batch
per core. No collectives; weights replicated to every core.

Layout strategy per core:
  - activations kept feature-major [D, S] for q/k (contraction over D_in),
    produced via PE-transpose of the [S, D] inputs.
  - v produced seq-major [S, D] directly (activations stationary), stored
    with a per-head ones column ([S, 12*(64+1)]) so the attention context
    matmul also produces the softmax denominator for free.
  - scores computed transposed sT[j, i] (lhsT = kT head slice, rhs = qT);
    heads processed in pairs on opposite PE row halves so their K=64
    matmuls run concurrently (row-group concurrency), each writing its own
    PSUM bank. One 2048-element exp per batch on ScalarE straight out of
    PSUM (no max-subtraction; scores are small). ctx^T = v_aug^T @ probsT
    with v_aug stationary (M=65 incl. the ones row), accumulated over j in
    per-head PSUM banks, then PE-transposed to seq-major and normalized.
  - the PE array is kept dense (HAM stays un-throttled) by emitting
    independent dense matmul streams inside the ACT-bound attention
    stretches: mol projections during pp, fc(prot) during mm,
    out-projection(prot) during pm.
  - fc over sequence: lhsT = cat tiles (seq-major), rhs = Wfc -> out_fcT
    feature-major; out projection: lhsT = out_fcT, rhs = Wout -> seq-major
    result, DMA'd out contiguously.
  - all matmul operands are float32r (full-speed fp32 mode, ~3e-4 rel).
"""

import contextlib

import numpy as np

import concourse.bass as bass
import concourse.mybir as mybir
import concourse.tile as tile
from concourse import bacc
from concourse import bass_utils
from concourse.masks import make_identity

F32 = mybir.dt.float32
F32R = mybir.dt.float32r
AF = mybir.ActivationFunctionType

B, P, M, D, H, DH = 8, 1024, 256, 768, 12, 64
S = P + M           # 1280
DT = D // 128       # 6 d-tiles
PT = P // 128       # 8
MT = M // 128       # 2
ST = S // 128       # 10
N_CORES = 8

_W_NAMES = ["Wq", "Wk", "Wv", "Wqm", "Wkm", "Wvm", "Wout", "Wout_mol"]
_B_NAMES = ["bq", "bk", "bv", "bqm", "bkm", "bvm", "bout", "bout_mol"]


def _chunks(total, size):
    out = []
    o = 0
    while o < total:
        out.append((o, min(size, total - o)))
        o += size
    return out


def _build():
    nc = bacc.Bacc("TRN2", target_bir_lowering=False, debug=False,
                   num_devices=N_CORES)

    io = {}
    io["hidden_states"] = nc.dram_tensor("hidden_states", [P, D], F32,
                                         kind="ExternalInput")
    io["mol"] = nc.dram_tensor("mol", [M, D], F32, kind="ExternalInput")
    for w in _W_NAMES:
        io[w] = nc.dram_tensor(w, [D, D], F32, kind="ExternalInput")
    for b in _B_NAMES:
        io[b] = nc.dram_tensor(b, [D], F32, kind="ExternalInput")
    io["Wfc"] = nc.dram_tensor("Wfc", [S, P], F32, kind="ExternalInput")
    io["bfc"] = nc.dram_tensor("bfc", [P], F32, kind="ExternalInput")
    io["Wfc_mol"] = nc.dram_tensor("Wfc_mol", [S, M], F32, kind="ExternalInput")
    io["bfc_mol"] = nc.dram_tensor("bfc_mol", [M], F32, kind="ExternalInput")
    io["out_prot"] = nc.dram_tensor("out_prot", [P, D], F32,
                                    kind="ExternalOutput")
    io["out_mol"] = nc.dram_tensor("out_mol", [M, D], F32,
                                   kind="ExternalOutput")
    # DRAM scratch for the concatenated attention contexts (seq-major).
    cat_prot = nc.dram_tensor("cat_prot", [S, D], F32R, kind="Internal")
    cat_mol = nc.dram_tensor("cat_mol", [S, D], F32R, kind="Internal")

    with tile.TileContext(nc) as tc:
        _kernel(tc, io, cat_prot, cat_mol)
    nc.compile()
    return nc


def _kernel(tc, io, cat_prot, cat_mol):
    nc = tc.nc
    ap = {k: v.ap() for k, v in io.items()}
    catp = cat_prot.ap().rearrange("(t p) d -> p t d", p=128)
    catm = cat_mol.ap().rearrange("(t p) d -> p t d", p=128)

    ctx = contextlib.ExitStack()
    with ctx:
        const = ctx.enter_context(tc.tile_pool(name="const", bufs=1))
        psA = ctx.enter_context(tc.tile_pool(name="psA", bufs=2, space="PSUM"))
        psS = ctx.enter_context(tc.tile_pool(name="psS", bufs=1, space="PSUM"))
        psC = ctx.enter_context(tc.tile_pool(name="psC", bufs=2, space="PSUM"))

        ident = const.tile([128, 128], F32)
        make_identity(nc, ident[:])

        def bcast(name, n):
            t = const.tile([128, n], F32, name=f"bc_{name}")
            src = ap[name].rearrange("(o n) -> o n", o=1).to_broadcast([128, n])
            nc.sync.dma_start(t[:], src)
            return t

        def ppart(name):
            t = const.tile([128, DT], F32, name=f"pp_{name}")
            nc.sync.dma_start(t[:], ap[name].rearrange("(mo p) -> p mo", p=128))
            return t

        ones_c = const.tile([128, H], F32, name="ones_c")
        nc.vector.memset(ones_c[:], 1.0)

        # long-lived activations
        actsQ = ctx.enter_context(tc.tile_pool(name="actsQ", bufs=1))
        qT = actsQ.tile([128, DT, P], F32R)
        qmT = actsQ.tile([128, DT, M], F32R)
        kmT = actsQ.tile([128, DT, M], F32R)
        vm_aug = actsQ.tile([128, MT, H * (DH + 1)], F32R)   # [128, 2, 780]

        actsK_cm = tc.tile_pool(name="actsK", bufs=1)
        actsK = actsK_cm.__enter__()
        kT = actsK.tile([128, DT, P], F32R)
        v_aug = actsK.tile([128, PT, H * (DH + 1)], F32R)    # [128, 8, 780]

        wp_pool = [None]
        stages = []

        # ---------- shared helpers -------------------------------------
        def project_fm(wname, bias_p, dstT, srcT, n_size):
            """Feature-major projection dstT[dout, s] = (x @ W + b)^T."""
            w_sb = wp_pool[0].tile([128, DT, D], F32R, tag="w", name=wname)
            nc.gpsimd.dma_start(
                w_sb[:], ap[wname].rearrange("(ko p) n -> p ko n", p=128))
            for mo in range(DT):
                for (n0, nsz) in _chunks(n_size, 512):
                    ps = psA.tile([128, 512], F32, tag="psA")
                    for ko in range(DT):
                        nc.tensor.matmul(
                            ps[:, 0:nsz],
                            w_sb[:, ko, mo * 128:(mo + 1) * 128],
                            srcT[:, ko, n0:n0 + nsz],
                            start=(ko == 0), stop=(ko == DT - 1))
                    nc.vector.tensor_scalar_add(
                        dstT[:, mo, n0:n0 + nsz], ps[:, 0:nsz],
                        bias_p[:, mo:mo + 1])

        def project_v(wname, bias_b, dst, srcT, seq_t):
            """Seq-major v projection into the ones-augmented layout."""
            w_sb = wp_pool[0].tile([128, DT, D], F32R, tag="w", name=wname)
            nc.gpsimd.dma_start(
                w_sb[:], ap[wname].rearrange("(ko p) n -> p ko n", p=128))
            for st in range(seq_t):
                for (n0, nsz) in _chunks(D, 512):
                    ps = psA.tile([128, 512], F32, tag="psA")
                    for ko in range(DT):
                        nc.tensor.matmul(
                            ps[:, 0:nsz],
                            srcT[:, ko, st * 128:(st + 1) * 128],
                            w_sb[:, ko, n0:n0 + nsz],
                            start=(ko == 0), stop=(ko == DT - 1))
                    h0, hn = n0 // DH, nsz // DH
                    dst_v = dst[:, st].rearrange(
                        "p (h x) -> p h x", x=DH + 1)[:, h0:h0 + hn, 0:DH]
                    nc.vector.tensor_add(
                        dst_v,
                        ps[:, 0:nsz].rearrange("p (h x) -> p h x", x=DH),
                        bias_b[:, n0:n0 + nsz].rearrange(
                            "p (h x) -> p h x", x=DH))
            for st in range(seq_t):
                nc.vector.tensor_copy(
                    dst[:, st].rearrange(
                        "p (h x) -> p h x", x=DH + 1)[:, :, DH],
                    ones_c[:])

        def attention(qsrc, SQ, ksrc, SK, vaug, cat_dst, st_base,
                      tagsfx, at, stages):
            JT = SK // 128
            CH = 512 if SQ >= 512 else SQ
            G = 2   # jt per exp batch
            nst = CH // 128
            nstg = (CH + 255) // 256  # stage tiles per chunk
            with nc.named_scope(f"att_{tagsfx}"):
                for (i0, _) in _chunks(SQ, CH):
                    ic = i0 // CH
                    for hp in range(H // 2):
                        h0, h1 = 2 * hp, 2 * hp + 1
                        ps_ct0 = psC.tile([128, CH], F32, tag="psC",
                                          name="ps_ct0")
                        ps_ct1 = psC.tile([128, CH], F32, tag="psC",
                                          name="ps_ct1")
                        for jg in range(0, JT, G):
                            assert jg + G <= JT
                            ps_sA = psS.tile([128, G, CH], F32, tag="psSA",
                                             name="ps_sA")
                            ps_sB = psS.tile([128, G, CH], F32, tag="psSB",
                                             name="ps_sB")
                            # score streak; the head pair runs concurrently
                            # on opposite PE row halves / separate banks
                            for g in range(G):
                                jt = jg + g
                                nc.tensor.matmul(
                                    ps_sA[:, g],
                                    ksrc[0:DH, hp, jt * 128:(jt + 1) * 128],
                                    qsrc[0:DH, hp, i0:i0 + CH],
                                    start=True, stop=True)
                                nc.tensor.matmul(
                                    ps_sB[:, g],
                                    ksrc[DH:128, hp, jt * 128:(jt + 1) * 128],
                                    qsrc[DH:128, hp, i0:i0 + CH],
                                    start=True, stop=True)
                            prA = at.tile([128, G, CH], F32R,
                                          tag=f"pr{tagsfx}A", name="prA")
                            prB = at.tile([128, G, CH], F32R,
                                          tag=f"pr{tagsfx}B", name="prB")
                            nc.scalar.activation(prA[:], ps_sA[:], AF.Exp,
                                                 scale=0.125)
                            nc.scalar.activation(prB[:], ps_sB[:], AF.Exp,
                                                 scale=0.125)
                            # ctx streak (K=128, M=65, N=CH)
                            for g in range(G):
                                jt = jg + g
                                nc.tensor.matmul(
                                    ps_ct0[0:DH + 1, :],
                                    vaug[:, jt,
                                         h0 * (DH + 1):(h0 + 1) * (DH + 1)],
                                    prA[:, g],
                                    start=(jt == 0), stop=(jt == JT - 1))
                                nc.tensor.matmul(
                                    ps_ct1[0:DH + 1, :],
                                    vaug[:, jt,
                                         h1 * (DH + 1):(h1 + 1) * (DH + 1)],
                                    prB[:, g],
                                    start=(jt == 0), stop=(jt == JT - 1))
                        # tail: seq-major transpose + normalize
                        for (h, ps_ct) in ((h0, ps_ct0), (h1, ps_ct1)):
                            ctx_sb = at.tile([DH + 1, CH], F32, tag="ctx")
                            nc.vector.tensor_copy(ctx_sb[:], ps_ct[0:DH + 1, :])
                            nc.vector.reciprocal(ctx_sb[DH:DH + 1, :],
                                                 ctx_sb[DH:DH + 1, :])
                            for ii in range(nst):
                                ps_t = psA.tile([128, 512], F32, tag="psA")
                                nc.tensor.transpose(
                                    ps_t[:, 0:DH + 1],
                                    ctx_sb[:, ii * 128:(ii + 1) * 128],
                                    ident[0:DH + 1, 0:DH + 1])
                                stage = stages[ic * nstg + ii // 2]
                                nc.vector.tensor_scalar_mul(
                                    stage[:, ii % 2, h * DH:(h + 1) * DH],
                                    ps_t[:, 0:DH], ps_t[:, DH:DH + 1])
                    # flush this chunk's stage tiles
                    for st2 in range(nstg):
                        st = st_base + (i0 // 128) + 2 * st2
                        stage = stages[ic * nstg + st2]
                        nc.sync.dma_start(cat_dst[:, st:st + 2, :], stage[:])

        def make_stages(at_pool, SQ, sfx):
            return [at_pool.tile([128, 2, D], F32R, tag="stage", bufs=2,
                                 name=f"stage_{sfx}{i}")
                    for i in range((SQ + 255) // 256)]

        def fc_stage(cat_src, wname, bias_bc, dstT, NP, pool):
            wfc_sb = pool.tile([128, ST, NP], F32R, name=f"sb_{wname}")
            nc.gpsimd.dma_start(
                wfc_sb[:], ap[wname].rearrange("(ko p) n -> p ko n", p=128))
            cat_sb = pool.tile([128, ST, D], F32R, name=f"cat_{wname}")
            for st in range(ST):
                nc.sync.dma_start(cat_sb[:, st], cat_src[:, st])
            for mo in range(DT):
                for (n0, nsz) in _chunks(NP, 512):
                    ps = psA.tile([128, 512], F32, tag="psA")
                    for st in range(ST):
                        nc.tensor.matmul(
                            ps[:, 0:nsz],
                            cat_sb[:, st, mo * 128:(mo + 1) * 128],
                            wfc_sb[:, st, n0:n0 + nsz],
                            start=(st == 0), stop=(st == ST - 1))
                    nc.vector.tensor_add(
                        dstT[:, mo, n0:n0 + nsz], ps[:, 0:nsz],
                        bias_bc[:, n0:n0 + nsz])

        def outproj(srcT, wname, bias_bc, out_dram, n_tiles, op, ost):
            wo_sb = op.tile([128, DT, D], F32R, name=f"sb_{wname}")
            nc.gpsimd.dma_start(
                wo_sb[:], ap[wname].rearrange("(ko p) n -> p ko n", p=128))
            for mo in range(n_tiles):
                o_sb = ost.tile([128, D], F32, tag="osb")
                for (n0, nsz) in _chunks(D, 512):
                    ps = psA.tile([128, 512], F32, tag="psA")
                    for kt in range(DT):
                        nc.tensor.matmul(
                            ps[:, 0:nsz],
                            srcT[:, kt, mo * 128:(mo + 1) * 128],
                            wo_sb[:, kt, n0:n0 + nsz],
                            start=(kt == 0), stop=(kt == DT - 1))
                    nc.vector.tensor_add(
                        o_sb[:, n0:n0 + nsz], ps[:, 0:nsz],
                        bias_bc[:, n0:n0 + nsz])
                nc.sync.dma_start(
                    out_dram[mo * 128:(mo + 1) * 128, :], o_sb[:])

        # ---- phase 1: input transposes --------------------------------
        proj_cm = tc.tile_pool(name="proj", bufs=1)
        proj = proj_cm.__enter__()
        molT = proj.tile([128, DT, M], F32R)
        hst_cm = tc.tile_pool(name="hst", bufs=1)
        hstp = hst_cm.__enter__()
        hsT = hstp.tile([128, DT, P], F32R)
        with tc.tile_pool(name="ld", bufs=3) as ld, \
                nc.named_scope("transpose_in"):
            for (src, dstT, nt) in ((ap["hidden_states"], hsT, PT),
                                    (ap["mol"], molT, MT)):
                for st in range(nt):
                    xs = ld.tile([128, D], F32, tag="xs")
                    nc.sync.dma_start(xs[:], src[st * 128:(st + 1) * 128, :])
                    for dt in range(DT):
                        pt = psA.tile([128, 512], F32, tag="psA")
                        nc.tensor.transpose(
                            pt[:, 0:128], xs[:, dt * 128:(dt + 1) * 128],
                            ident[:])
                        nc.vector.tensor_copy(
                            dstT[:, dt, st * 128:(st + 1) * 128],
                            pt[:, 0:128])

        # bias tiles
        bq_p, bk_p = ppart("bq"), ppart("bk")
        bqm_p, bkm_p = ppart("bqm"), ppart("bkm")
        bv_b = bcast("bv", D)
        bvm_b = bcast("bvm", D)
        bfc_b = bcast("bfc", P)
        bfcm_b = bcast("bfc_mol", M)
        bout_b = bcast("bout", D)
        boutm_b = bcast("bout_mol", D)

        # ---- phase 2a: protein projections ----------------------------
        wp1_cm = tc.tile_pool(name="wp1", bufs=2)
        wp_pool[0] = wp1_cm.__enter__()
        with nc.named_scope("proj"):
            project_fm("Wq", bq_p, qT, hsT, P)
            project_fm("Wk", bk_p, kT, hsT, P)
            project_v("Wv", bv_b, v_aug, hsT, PT)
        wp1_cm.__exit__(None, None, None)
        hst_cm.__exit__(None, None, None)

        # ---- pp + pm attention, mol projections as PE filler ----------
        wp2_cm = tc.tile_pool(name="wp2", bufs=2)
        wp_pool[0] = wp2_cm.__enter__()
        atb_cm = tc.tile_pool(name="atb", bufs=2)
        atb = atb_cm.__enter__()
        st_pp = make_stages(atb, P, "pp")
        attention(qT, P, kT, P, v_aug, catp, 0, "pp", atb, st_pp)
        with nc.named_scope("proj_mol"):
            project_fm("Wqm", bqm_p, qmT, molT, M)
            project_fm("Wkm", bkm_p, kmT, molT, M)
            project_v("Wvm", bvm_b, vm_aug, molT, MT)
        st_pm = make_stages(atb, P, "pm")
        attention(qT, P, kmT, M, vm_aug, catm, MT, "pm", atb, st_pm)
        st_mp = make_stages(atb, M, "mp")
        attention(qmT, M, kT, P, v_aug, catp, PT, "mp", atb, st_mp)
        atb_cm.__exit__(None, None, None)
        wp2_cm.__exit__(None, None, None)
        proj_cm.__exit__(None, None, None)
        actsK_cm.__exit__(None, None, None)

        # ---- fc(prot) overlapping mm attention ------------------------
        acts2 = ctx.enter_context(tc.tile_pool(name="acts2", bufs=1))
        ofcT = acts2.tile([128, DT, P], F32R)
        ofcmT = acts2.tile([128, DT, M], F32R)

        fcp_cm = tc.tile_pool(name="fcp", bufs=1)
        fcp = fcp_cm.__enter__()
        with nc.named_scope("fc_prot"):
            fc_stage(catp, "Wfc", bfc_b, ofcT, P, fcp)

        atm_cm = tc.tile_pool(name="atm", bufs=2)
        atm = atm_cm.__enter__()
        st_mm = make_stages(atm, M, "mm")
        attention(qmT, M, kmT, M, vm_aug, catm, 0, "mm", atm, st_mm)
        atm_cm.__exit__(None, None, None)
        fcp_cm.__exit__(None, None, None)

        # ---- fc(mol) + outprojections tail ----------------------------
        op = ctx.enter_context(tc.tile_pool(name="op", bufs=1))
        ost = ctx.enter_context(tc.tile_pool(name="ost", bufs=3))
        with tc.tile_pool(name="fcm", bufs=1) as fcm, nc.named_scope("fc_mol"):
            fc_stage(catm, "Wfc_mol", bfcm_b, ofcmT, M, fcm)
            with nc.named_scope("outproj_prot"):
                outproj(ofcT, "Wout", bout_b, ap["out_prot"], PT, op, ost)

        with nc.named_scope("outproj_mol"):
            outproj(ofcmT, "Wout_mol", boutm_b, ap["out_mol"], MT, op, ost)


_NC_CACHE = None


def _get_program():
    global _NC_CACHE
    if _NC_CACHE is None:
        _NC_CACHE = _build()
    return _NC_CACHE


def kernel(**inputs):
    nc = _get_program()
    per_core_names = (["hidden_states", "mol"] + _W_NAMES + _B_NAMES
                      + ["Wfc", "bfc", "Wfc_mol", "bfc_mol"])
    in_maps = []
    for c in range(N_CORES):
        m = {}
        for name in per_core_names:
            arr = np.ascontiguousarray(np.asarray(inputs[name], dtype=np.float32))
            if name in ("hidden_states", "mol"):
                arr = arr[c]
            m[name] = arr
        in_maps.append(m)

    res = bass_utils.run_bass_kernel_spmd(nc, in_maps,
                                          core_ids=list(range(N_CORES)))
    global LAST_RESULTS
    LAST_RESULTS = res
    out_prot = np.stack([res.results[c]["out_prot"] for c in range(N_CORES)])
    out_mol = np.stack([res.results[c]["out_mol"] for c in range(N_CORES)])
    return out_prot, out_mol


LAST_RESULTS = None


# revision 25
# speedup vs baseline: 1.2689x; 1.2689x over previous
"""Trainium2 Bass kernel for nn_Attention_77146202570808.

Dual-stream (protein/molecule) multi-head attention block:
  q/k/v projections for both streams, 4 attention passes (pp, mm, pm, mp),
  a Linear over the *sequence* axis (P+M -> P / M), and output projections.

Sharding: data-parallel over batch. B=8 batches, 8 NeuronCores, one batch
per core. No collectives; weights replicated to every core.

Layout strategy per core:
  - activations kept feature-major [D, S] for q/k (contraction over D_in),
    produced via PE-transpose of the [S, D] inputs.
  - v produced seq-major [S, D] directly (activations stationary), stored
    with a per-head ones column ([S, 12*(64+1)]) so the attention context
    matmul also produces the softmax denominator for free.
  - scores computed transposed sT[j, i] (lhsT = kT head slice, rhs = qT);
    heads processed in pairs on opposite PE row halves so their K=64
    matmuls run concurrently (row-group concurrency), each writing its own
    PSUM bank. One 2048-element exp per batch on ScalarE straight out of
    PSUM (no max-subtraction; scores are small). ctx^T = v_aug^T @ probsT
    with v_aug stationary (M=65 incl. the ones row), accumulated over j in
    per-head PSUM banks, then PE-transposed to seq-major and normalized.
  - the PE array is kept dense (HAM stays un-throttled) by emitting
    independent dense matmul streams inside the ACT-bound attention
    stretches: mol projections during pp, fc(prot) during mm,
    out-projection(prot) during pm.
  - fc over sequence: lhsT = cat tiles (seq-major), rhs = Wfc -> out_fcT
    feature-major; out projection: lhsT = out_fcT, rhs = Wout -> seq-major
    result, DMA'd out contiguously.
  - all matmul operands are float32r (full-speed fp32 mode, ~3e-4 rel).
"""

import contextlib

import numpy as np

import concourse.bass as bass
import concourse.mybir as mybir
import concourse.tile as tile
from concourse import bacc
from concourse import bass_utils
from concourse.masks import make_identity

F32 = mybir.dt.float32
F32R = mybir.dt.float32r
AF = mybir.ActivationFunctionType

B, P, M, D, H, DH = 8, 1024, 256, 768, 12, 64
S = P + M           # 1280
DT = D // 128       # 6 d-tiles
PT = P // 128       # 8
MT = M // 128       # 2
ST = S // 128       # 10
N_CORES = 8

_W_NAMES = ["Wq", "Wk", "Wv", "Wqm", "Wkm", "Wvm", "Wout", "Wout_mol"]
_B_NAMES = ["bq", "bk", "bv", "bqm", "bkm", "bvm", "bout", "bout_mol"]


def _chunks(total, size):
    out = []
    o = 0
    while o < total:
        out.append((o, min(size, total - o)))
        o += size
    return out


def _build():
    nc = bacc.Bacc("TRN2", target_bir_lowering=False, debug=False,
                   num_devices=N_CORES)

    io = {}
    io["hidden_states"] = nc.dram_tensor("hidden_states", [P, D], F32,
                                         kind="ExternalInput")
    io["mol"] = nc.dram_tensor("mol", [M, D], F32, kind="ExternalInput")
    for w in _W_NAMES:
        io[w] = nc.dram_tensor(w, [D, D], F32, kind="ExternalInput")
    for b in _B_NAMES:
        io[b] = nc.dram_tensor(b, [D], F32, kind="ExternalInput")
    io["Wfc"] = nc.dram_tensor("Wfc", [S, P], F32, kind="ExternalInput")
    io["bfc"] = nc.dram_tensor("bfc", [P], F32, kind="ExternalInput")
    io["Wfc_mol"] = nc.dram_tensor("Wfc_mol", [S, M], F32, kind="ExternalInput")
    io["bfc_mol"] = nc.dram_tensor("bfc_mol", [M], F32, kind="ExternalInput")
    io["out_prot"] = nc.dram_tensor("out_prot", [P, D], F32,
                                    kind="ExternalOutput")
    io["out_mol"] = nc.dram_tensor("out_mol", [M, D], F32,
                                   kind="ExternalOutput")
    # DRAM scratch for the concatenated attention contexts (seq-major).
    cat_prot = nc.dram_tensor("cat_prot", [S, D], F32R, kind="Internal")
    cat_mol = nc.dram_tensor("cat_mol", [S, D], F32R, kind="Internal")

    with tile.TileContext(nc) as tc:
        _kernel(tc, io, cat_prot, cat_mol)
    nc.compile()
    return nc


def _kernel(tc, io, cat_prot, cat_mol):
    nc = tc.nc
    ap = {k: v.ap() for k, v in io.items()}
    catp = cat_prot.ap().rearrange("(t p) d -> p t d", p=128)
    catm = cat_mol.ap().rearrange("(t p) d -> p t d", p=128)

    ctx = contextlib.ExitStack()
    with ctx:
        const = ctx.enter_context(tc.tile_pool(name="const", bufs=1))
        psA = ctx.enter_context(tc.tile_pool(name="psA", bufs=2, space="PSUM"))
        psS = ctx.enter_context(tc.tile_pool(name="psS", bufs=1, space="PSUM"))
        psC = ctx.enter_context(tc.tile_pool(name="psC", bufs=2, space="PSUM"))

        ident = const.tile([128, 128], F32)
        make_identity(nc, ident[:])

        def bcast(name, n):
            t = const.tile([128, n], F32, name=f"bc_{name}")
            src = ap[name].rearrange("(o n) -> o n", o=1).to_broadcast([128, n])
            nc.sync.dma_start(t[:], src)
            return t

        def ppart(name):
            t = const.tile([128, DT], F32, name=f"pp_{name}")
            nc.sync.dma_start(t[:], ap[name].rearrange("(mo p) -> p mo", p=128))
            return t

        ones_c = const.tile([128, H], F32, name="ones_c")
        nc.vector.memset(ones_c[:], 1.0)

        # long-lived activations
        actsQ = ctx.enter_context(tc.tile_pool(name="actsQ", bufs=1))
        qT = actsQ.tile([128, DT, P], F32R)
        qmT = actsQ.tile([128, DT, M], F32R)
        kmT = actsQ.tile([128, DT, M], F32R)
        vm_aug = actsQ.tile([128, MT, H * (DH + 1)], F32R)   # [128, 2, 780]

        actsK_cm = tc.tile_pool(name="actsK", bufs=1)
        actsK = actsK_cm.__enter__()
        kT = actsK.tile([128, DT, P], F32R)
        v_aug = actsK.tile([128, PT, H * (DH + 1)], F32R)    # [128, 8, 780]

        wp_pool = [None]
        stages = []

        # ---------- shared helpers -------------------------------------
        def project_fm(wname, bias_p, dstT, srcT, n_size):
            """Feature-major projection dstT[dout, s] = (x @ W + b)^T."""
            w_sb = wp_pool[0].tile([128, DT, D], F32R, tag="w", name=wname)
            nc.gpsimd.dma_start(
                w_sb[:], ap[wname].rearrange("(ko p) n -> p ko n", p=128))
            for mo in range(DT):
                for (n0, nsz) in _chunks(n_size, 512):
                    ps = psA.tile([128, 512], F32, tag="psA")
                    for ko in range(DT):
                        nc.tensor.matmul(
                            ps[:, 0:nsz],
                            w_sb[:, ko, mo * 128:(mo + 1) * 128],
                            srcT[:, ko, n0:n0 + nsz],
                            start=(ko == 0), stop=(ko == DT - 1))
                    nc.vector.tensor_scalar_add(
                        dstT[:, mo, n0:n0 + nsz], ps[:, 0:nsz],
                        bias_p[:, mo:mo + 1])

        def project_v(wname, bias_b, dst, srcT, seq_t):
            """Seq-major v projection into the ones-augmented layout."""
            w_sb = wp_pool[0].tile([128, DT, D], F32R, tag="w", name=wname)
            nc.gpsimd.dma_start(
                w_sb[:], ap[wname].rearrange("(ko p) n -> p ko n", p=128))
            for st in range(seq_t):
                for (n0, nsz) in _chunks(D, 512):
                    ps = psA.tile([128, 512], F32, tag="psA")
                    for ko in range(DT):
                        nc.tensor.matmul(
                            ps[:, 0:nsz],
                            srcT[:, ko, st * 128:(st + 1) * 128],
                            w_sb[:, ko, n0:n0 + nsz],
                            start=(ko == 0), stop=(ko == DT - 1))
                    h0, hn = n0 // DH, nsz // DH
                    dst_v = dst[:, st].rearrange(
                        "p (h x) -> p h x", x=DH + 1)[:, h0:h0 + hn, 0:DH]
                    nc.vector.tensor_add(
                        dst_v,
                        ps[:, 0:nsz].rearrange("p (h x) -> p h x", x=DH),
                        bias_b[:, n0:n0 + nsz].rearrange(
                            "p (h x) -> p h x", x=DH))
            for st in range(seq_t):
                nc.vector.tensor_copy(
                    dst[:, st].rearrange(
                        "p (h x) -> p h x", x=DH + 1)[:, :, DH],
                    ones_c[:])

        def attention(qsrc, SQ, ksrc, SK, vaug, cat_dst, st_base,
                      tagsfx, at, stages):
            JT = SK // 128
            CH = 512 if SQ >= 512 else SQ
            G = 2   # jt per exp batch
            nst = CH // 128
            nstg = (CH + 255) // 256  # stage tiles per chunk
            with nc.named_scope(f"att_{tagsfx}"):
                for (i0, _) in _chunks(SQ, CH):
                    ic = i0 // CH
                    for hp in range(H // 2):
                        h0, h1 = 2 * hp, 2 * hp + 1
                        ps_ct0 = psC.tile([128, CH], F32, tag="psC",
                                          name="ps_ct0")
                        ps_ct1 = psC.tile([128, CH], F32, tag="psC",
                                          name="ps_ct1")
                        for jg in range(0, JT, G):
                            assert jg + G <= JT
                            ps_sA = psS.tile([128, G, CH], F32, tag="psSA",
                                             name="ps_sA")
                            ps_sB = psS.tile([128, G, CH], F32, tag="psSB",
                                             name="ps_sB")
                            # score streak; the head pair runs concurrently
                            # on opposite PE row halves / separate banks
                            for g in range(G):
                                jt = jg + g
                                nc.tensor.matmul(
                                    ps_sA[:, g],
                                    ksrc[0:DH, hp, jt * 128:(jt + 1) * 128],
                                    qsrc[0:DH, hp, i0:i0 + CH],
                                    start=True, stop=True)
                                nc.tensor.matmul(
                                    ps_sB[:, g],
                                    ksrc[DH:128, hp, jt * 128:(jt + 1) * 128],
                                    qsrc[DH:128, hp, i0:i0 + CH],
                                    start=True, stop=True)
                            prA = at.tile([128, G, CH], F32R,
                                          tag=f"pr{tagsfx}A", name="prA")
                            prB = at.tile([128, G, CH], F32R,
                                          tag=f"pr{tagsfx}B", name="prB")
                            nc.scalar.activation(prA[:], ps_sA[:], AF.Exp,
                                                 scale=0.125)
                            nc.scalar.activation(prB[:], ps_sB[:], AF.Exp,
                                                 scale=0.125)
                            # ctx streak (K=128, M=65, N=CH)
                            for g in range(G):
                                jt = jg + g
                                nc.tensor.matmul(
                                    ps_ct0[0:DH + 1, :],
                                    vaug[:, jt,
                                         h0 * (DH + 1):(h0 + 1) * (DH + 1)],
                                    prA[:, g],
                                    start=(jt == 0), stop=(jt == JT - 1))
                                nc.tensor.matmul(
                                    ps_ct1[0:DH + 1, :],
                                    vaug[:, jt,
                                         h1 * (DH + 1):(h1 + 1) * (DH + 1)],
                                    prB[:, g],
                                    start=(jt == 0), stop=(jt == JT - 1))
                        # tail: seq-major transpose + normalize
                        for (h, ps_ct) in ((h0, ps_ct0), (h1, ps_ct1)):
                            ctx_sb = at.tile([DH + 1, CH], F32, tag="ctx")
                            nc.vector.tensor_copy(ctx_sb[:], ps_ct[0:DH + 1, :])
                            for ii in range(nst):
                                ps_t = psA.tile([128, 512], F32, tag="psA")
                                nc.tensor.transpose(
                                    ps_t[:, 0:DH + 1],
                                    ctx_sb[:, ii * 128:(ii + 1) * 128],
                                    ident[0:DH + 1, 0:DH + 1])
                                rec = at.tile([128, 1], F32, tag="rec", bufs=4)
                                nc.vector.reciprocal(rec[:], ps_t[:, DH:DH + 1])
                                stage = stages[ic * nstg + ii // 2]
                                nc.vector.tensor_scalar_mul(
                                    stage[:, ii % 2, h * DH:(h + 1) * DH],
                                    ps_t[:, 0:DH], rec[:])
                    # flush this chunk's stage tiles
                    for st2 in range(nstg):
                        st = st_base + (i0 // 128) + 2 * st2
                        stage = stages[ic * nstg + st2]
                        nc.sync.dma_start(cat_dst[:, st:st + 2, :], stage[:])

        def make_stages(at_pool, SQ, sfx):
            return [at_pool.tile([128, 2, D], F32R, tag="stage", bufs=2,
                                 name=f"stage_{sfx}{i}")
                    for i in range((SQ + 255) // 256)]

        def fc_stage(cat_src, wname, bias_bc, dstT, NP, pool):
            wfc_sb = pool.tile([128, ST, NP], F32R, name=f"sb_{wname}")
            nc.gpsimd.dma_start(
                wfc_sb[:], ap[wname].rearrange("(ko p) n -> p ko n", p=128))
            cat_sb = pool.tile([128, ST, D], F32R, name=f"cat_{wname}")
            for st in range(ST):
                nc.sync.dma_start(cat_sb[:, st], cat_src[:, st])
            for mo in range(DT):
                for (n0, nsz) in _chunks(NP, 512):
                    ps = psA.tile([128, 512], F32, tag="psA")
                    for st in range(ST):
                        nc.tensor.matmul(
                            ps[:, 0:nsz],
                            cat_sb[:, st, mo * 128:(mo + 1) * 128],
                            wfc_sb[:, st, n0:n0 + nsz],
                            start=(st == 0), stop=(st == ST - 1))
                    nc.vector.tensor_add(
                        dstT[:, mo, n0:n0 + nsz], ps[:, 0:nsz],
                        bias_bc[:, n0:n0 + nsz])

        def outproj(srcT, wname, bias_bc, out_dram, n_tiles, op, ost):
            wo_sb = op.tile([128, DT, D], F32R, name=f"sb_{wname}")
            nc.gpsimd.dma_start(
                wo_sb[:], ap[wname].rearrange("(ko p) n -> p ko n", p=128))
            for mo in range(n_tiles):
                o_sb = ost.tile([128, D], F32, tag="osb")
                for (n0, nsz) in _chunks(D, 512):
                    ps = psA.tile([128, 512], F32, tag="psA")
                    for kt in range(DT):
                        nc.tensor.matmul(
                            ps[:, 0:nsz],
                            srcT[:, kt, mo * 128:(mo + 1) * 128],
                            wo_sb[:, kt, n0:n0 + nsz],
                            start=(kt == 0), stop=(kt == DT - 1))
                    nc.vector.tensor_add(
                        o_sb[:, n0:n0 + nsz], ps[:, 0:nsz],
                        bias_bc[:, n0:n0 + nsz])
                nc.sync.dma_start(
                    out_dram[mo * 128:(mo + 1) * 128, :], o_sb[:])

        # ---- phase 1: input transposes --------------------------------
        proj_cm = tc.tile_pool(name="proj", bufs=1)
        proj = proj_cm.__enter__()
        molT = proj.tile([128, DT, M], F32R)
        hst_cm = tc.tile_pool(name="hst", bufs=1)
        hstp = hst_cm.__enter__()
        hsT = hstp.tile([128, DT, P], F32R)
        with tc.tile_pool(name="ld", bufs=3) as ld, \
                nc.named_scope("transpose_in"):
            for (src, dstT, nt) in ((ap["hidden_states"], hsT, PT),
                                    (ap["mol"], molT, MT)):
                for st in range(nt):
                    xs = ld.tile([128, D], F32, tag="xs")
                    nc.sync.dma_start(xs[:], src[st * 128:(st + 1) * 128, :])
                    for dt in range(DT):
                        pt = psA.tile([128, 512], F32, tag="psA")
                        nc.tensor.transpose(
                            pt[:, 0:128], xs[:, dt * 128:(dt + 1) * 128],
                            ident[:])
                        nc.vector.tensor_copy(
                            dstT[:, dt, st * 128:(st + 1) * 128],
                            pt[:, 0:128])

        # bias tiles
        bq_p, bk_p = ppart("bq"), ppart("bk")
        bqm_p, bkm_p = ppart("bqm"), ppart("bkm")
        bv_b = bcast("bv", D)
        bvm_b = bcast("bvm", D)
        bfc_b = bcast("bfc", P)
        bfcm_b = bcast("bfc_mol", M)
        bout_b = bcast("bout", D)
        boutm_b = bcast("bout_mol", D)

        # ---- phase 2a: protein projections ----------------------------
        wp1_cm = tc.tile_pool(name="wp1", bufs=2)
        wp_pool[0] = wp1_cm.__enter__()
        with nc.named_scope("proj"):
            project_fm("Wq", bq_p, qT, hsT, P)
            project_fm("Wk", bk_p, kT, hsT, P)
            project_v("Wv", bv_b, v_aug, hsT, PT)
        wp1_cm.__exit__(None, None, None)
        hst_cm.__exit__(None, None, None)

        # ---- pp + pm attention, mol projections as PE filler ----------
        wp2_cm = tc.tile_pool(name="wp2", bufs=2)
        wp_pool[0] = wp2_cm.__enter__()
        atb_cm = tc.tile_pool(name="atb", bufs=2)
        atb = atb_cm.__enter__()
        st_pp = make_stages(atb, P, "pp")
        attention(qT, P, kT, P, v_aug, catp, 0, "pp", atb, st_pp)
        with nc.named_scope("proj_mol"):
            project_fm("Wqm", bqm_p, qmT, molT, M)
            project_fm("Wkm", bkm_p, kmT, molT, M)
            project_v("Wvm", bvm_b, vm_aug, molT, MT)
        st_pm = make_stages(atb, P, "pm")
        attention(qT, P, kmT, M, vm_aug, catm, MT, "pm", atb, st_pm)
        st_mp = make_stages(atb, M, "mp")
        attention(qmT, M, kT, P, v_aug, catp, PT, "mp", atb, st_mp)
        atb_cm.__exit__(None, None, None)
        wp2_cm.__exit__(None, None, None)
        proj_cm.__exit__(None, None, None)
        actsK_cm.__exit__(None, None, None)

        # ---- fc(prot) overlapping mm attention ------------------------
        acts2 = ctx.enter_context(tc.tile_pool(name="acts2", bufs=1))
        ofcT = acts2.tile([128, DT, P], F32R)
        ofcmT = acts2.tile([128, DT, M], F32R)

        fcp_cm = tc.tile_pool(name="fcp", bufs=1)
        fcp = fcp_cm.__enter__()
        with nc.named_scope("fc_prot"):
            fc_stage(catp, "Wfc", bfc_b, ofcT, P, fcp)

        atm_cm = tc.tile_pool(name="atm", bufs=2)
        atm = atm_cm.__enter__()
        st_mm = make_stages(atm, M, "mm")
        attention(qmT, M, kmT, M, vm_aug, catm, 0, "mm", atm, st_mm)
        atm_cm.__exit__(None, None, None)
        fcp_cm.__exit__(None, None, None)

        # ---- fc(mol) + outprojections tail ----------------------------
        op = ctx.enter_context(tc.tile_pool(name="op", bufs=1))
        ost = ctx.enter_context(tc.tile_pool(name="ost", bufs=3))
        with tc.tile_pool(name="fcm", bufs=1) as fcm, nc.named_scope("fc_mol"):
            fc_stage(catm, "Wfc_mol", bfcm_b, ofcmT, M, fcm)
            with nc.named_scope("outproj_prot"):
                outproj(ofcT, "Wout", bout_b, ap["out_prot"], PT, op, ost)

        with nc.named_scope("outproj_mol"):
            outproj(ofcmT, "Wout_mol", boutm_b, ap["out_mol"], MT, op, ost)


_NC_CACHE = None


def _get_program():
    global _NC_CACHE
    if _NC_CACHE is None:
        _NC_CACHE = _build()
    return _NC_CACHE


def kernel(**inputs):
    nc = _get_program()
    per_core_names = (["hidden_states", "mol"] + _W_NAMES + _B_NAMES
                      + ["Wfc", "bfc", "Wfc_mol", "bfc_mol"])
    in_maps = []
    for c in range(N_CORES):
        m = {}
        for name in per_core_names:
            arr = np.ascontiguousarray(np.asarray(inputs[name], dtype=np.float32))
            if name in ("hidden_states", "mol"):
                arr = arr[c]
            m[name] = arr
        in_maps.append(m)

    res = bass_utils.run_bass_kernel_spmd(nc, in_maps,
                                          core_ids=list(range(N_CORES)))
    global LAST_RESULTS
    LAST_RESULTS = res
    out_prot = np.stack([res.results[c]["out_prot"] for c in range(N_CORES)])
    out_mol = np.stack([res.results[c]["out_mol"] for c in range(N_CORES)])
    return out_prot, out_mol


LAST_RESULTS = None


# revision 26
# speedup vs baseline: 1.3121x; 1.0341x over previous
"""Trainium2 Bass kernel for nn_Attention_77146202570808.

Dual-stream (protein/molecule) multi-head attention block:
  q/k/v projections for both streams, 4 attention passes (pp, mm, pm, mp),
  a Linear over the *sequence* axis (P+M -> P / M), and output projections.

Sharding: data-parallel over batch. B=8 batches, 8 NeuronCores, one batch
per core. No collectives; weights replicated to every core.

Layout strategy per core:
  - activations kept feature-major [D, S] for q/k (contraction over D_in),
    produced via PE-transpose of the [S, D] inputs.
  - v produced seq-major [S, D] directly (activations stationary), stored
    with a per-head ones column ([S, 12*(64+1)]) so the attention context
    matmul also produces the softmax denominator for free.
  - scores computed transposed sT[j, i] (lhsT = kT head slice, rhs = qT);
    heads processed in pairs on opposite PE row halves so their K=64
    matmuls run concurrently (row-group concurrency), each writing its own
    PSUM bank. One 2048-element exp per batch on ScalarE straight out of
    PSUM (no max-subtraction; scores are small). ctx^T = v_aug^T @ probsT
    with v_aug stationary (M=65 incl. the ones row), accumulated over j in
    per-head PSUM banks, then PE-transposed to seq-major and normalized.
  - the PE array is kept dense (HAM stays un-throttled) by emitting
    independent dense matmul streams inside the ACT-bound attention
    stretches: mol projections during pp, fc(prot) during mm,
    out-projection(prot) during pm.
  - fc over sequence: lhsT = cat tiles (seq-major), rhs = Wfc -> out_fcT
    feature-major; out projection: lhsT = out_fcT, rhs = Wout -> seq-major
    result, DMA'd out contiguously.
  - all matmul operands are float32r (full-speed fp32 mode, ~3e-4 rel).
"""

import contextlib

import numpy as np

import concourse.bass as bass
import concourse.mybir as mybir
import concourse.tile as tile
from concourse import bacc
from concourse import bass_utils
from concourse.masks import make_identity

F32 = mybir.dt.float32
F32R = mybir.dt.float32r
AF = mybir.ActivationFunctionType

B, P, M, D, H, DH = 8, 1024, 256, 768, 12, 64
S = P + M           # 1280
DT = D // 128       # 6 d-tiles
PT = P // 128       # 8
MT = M // 128       # 2
ST = S // 128       # 10
N_CORES = 8

_W_NAMES = ["Wq", "Wk", "Wv", "Wqm", "Wkm", "Wvm", "Wout", "Wout_mol"]
_B_NAMES = ["bq", "bk", "bv", "bqm", "bkm", "bvm", "bout", "bout_mol"]


def _chunks(total, size):
    out = []
    o = 0
    while o < total:
        out.append((o, min(size, total - o)))
        o += size
    return out


def _build():
    nc = bacc.Bacc("TRN2", target_bir_lowering=False, debug=False,
                   num_devices=N_CORES)

    io = {}
    io["hidden_states"] = nc.dram_tensor("hidden_states", [P, D], F32,
                                         kind="ExternalInput")
    io["mol"] = nc.dram_tensor("mol", [M, D], F32, kind="ExternalInput")
    for w in _W_NAMES:
        io[w] = nc.dram_tensor(w, [D, D], F32, kind="ExternalInput")
    for b in _B_NAMES:
        io[b] = nc.dram_tensor(b, [D], F32, kind="ExternalInput")
    io["Wfc"] = nc.dram_tensor("Wfc", [S, P], F32, kind="ExternalInput")
    io["bfc"] = nc.dram_tensor("bfc", [P], F32, kind="ExternalInput")
    io["Wfc_mol"] = nc.dram_tensor("Wfc_mol", [S, M], F32, kind="ExternalInput")
    io["bfc_mol"] = nc.dram_tensor("bfc_mol", [M], F32, kind="ExternalInput")
    io["out_prot"] = nc.dram_tensor("out_prot", [P, D], F32,
                                    kind="ExternalOutput")
    io["out_mol"] = nc.dram_tensor("out_mol", [M, D], F32,
                                   kind="ExternalOutput")
    # DRAM scratch for the concatenated attention contexts (seq-major).
    cat_prot = nc.dram_tensor("cat_prot", [S, D], F32R, kind="Internal")
    cat_mol = nc.dram_tensor("cat_mol", [S, D], F32R, kind="Internal")

    with tile.TileContext(nc) as tc:
        _kernel(tc, io, cat_prot, cat_mol)
    nc.compile()
    return nc


def _kernel(tc, io, cat_prot, cat_mol):
    nc = tc.nc
    ap = {k: v.ap() for k, v in io.items()}
    catp = cat_prot.ap().rearrange("(t p) d -> p t d", p=128)
    catm = cat_mol.ap().rearrange("(t p) d -> p t d", p=128)

    ctx = contextlib.ExitStack()
    with ctx:
        const = ctx.enter_context(tc.tile_pool(name="const", bufs=1))
        psA = ctx.enter_context(tc.tile_pool(name="psA", bufs=2, space="PSUM"))
        psS = ctx.enter_context(tc.tile_pool(name="psS", bufs=1, space="PSUM"))
        psC = ctx.enter_context(tc.tile_pool(name="psC", bufs=2, space="PSUM"))

        ident = const.tile([128, 128], F32)
        make_identity(nc, ident[:])

        def bcast(name, n):
            t = const.tile([128, n], F32, name=f"bc_{name}")
            src = ap[name].rearrange("(o n) -> o n", o=1).to_broadcast([128, n])
            nc.sync.dma_start(t[:], src)
            return t

        def ppart(name):
            t = const.tile([128, DT], F32, name=f"pp_{name}")
            nc.sync.dma_start(t[:], ap[name].rearrange("(mo p) -> p mo", p=128))
            return t

        ones_c = const.tile([128, H], F32, name="ones_c")
        nc.vector.memset(ones_c[:], 1.0)

        # long-lived activations
        actsQ = ctx.enter_context(tc.tile_pool(name="actsQ", bufs=1))
        qT = actsQ.tile([128, DT, P], F32R)
        qmT = actsQ.tile([128, DT, M], F32R)
        kmT = actsQ.tile([128, DT, M], F32R)
        vm_aug = actsQ.tile([128, MT, H * (DH + 1)], F32R)   # [128, 2, 780]

        actsK_cm = tc.tile_pool(name="actsK", bufs=1)
        actsK = actsK_cm.__enter__()
        kT = actsK.tile([128, DT, P], F32R)
        v_aug = actsK.tile([128, PT, H * (DH + 1)], F32R)    # [128, 8, 780]

        wp_pool = [None]
        stages = []

        # ---------- shared helpers -------------------------------------
        def project_fm(wname, bias_p, dstT, srcT, n_size):
            """Feature-major projection dstT[dout, s] = (x @ W + b)^T."""
            w_sb = wp_pool[0].tile([128, DT, D], F32R, tag="w", name=wname)
            nc.gpsimd.dma_start(
                w_sb[:], ap[wname].rearrange("(ko p) n -> p ko n", p=128))
            for mo in range(DT):
                for (n0, nsz) in _chunks(n_size, 512):
                    ps = psA.tile([128, 512], F32, tag="psA")
                    for ko in range(DT):
                        nc.tensor.matmul(
                            ps[:, 0:nsz],
                            w_sb[:, ko, mo * 128:(mo + 1) * 128],
                            srcT[:, ko, n0:n0 + nsz],
                            start=(ko == 0), stop=(ko == DT - 1))
                    nc.vector.tensor_scalar_add(
                        dstT[:, mo, n0:n0 + nsz], ps[:, 0:nsz],
                        bias_p[:, mo:mo + 1])

        def project_v(wname, bias_b, dst, srcT, seq_t):
            """Seq-major v projection into the ones-augmented layout."""
            w_sb = wp_pool[0].tile([128, DT, D], F32R, tag="w", name=wname)
            nc.gpsimd.dma_start(
                w_sb[:], ap[wname].rearrange("(ko p) n -> p ko n", p=128))
            for st in range(seq_t):
                for (n0, nsz) in _chunks(D, 512):
                    ps = psA.tile([128, 512], F32, tag="psA")
                    for ko in range(DT):
                        nc.tensor.matmul(
                            ps[:, 0:nsz],
                            srcT[:, ko, st * 128:(st + 1) * 128],
                            w_sb[:, ko, n0:n0 + nsz],
                            start=(ko == 0), stop=(ko == DT - 1))
                    h0, hn = n0 // DH, nsz // DH
                    dst_v = dst[:, st].rearrange(
                        "p (h x) -> p h x", x=DH + 1)[:, h0:h0 + hn, 0:DH]
                    nc.vector.tensor_add(
                        dst_v,
                        ps[:, 0:nsz].rearrange("p (h x) -> p h x", x=DH),
                        bias_b[:, n0:n0 + nsz].rearrange(
                            "p (h x) -> p h x", x=DH))
            for st in range(seq_t):
                nc.vector.tensor_copy(
                    dst[:, st].rearrange(
                        "p (h x) -> p h x", x=DH + 1)[:, :, DH],
                    ones_c[:])

        def attention(qsrc, SQ, ksrc, SK, vaug, cat_dst, st_base,
                      tagsfx, at, stages):
            JT = SK // 128
            CH = 512 if SQ >= 512 else SQ
            G = 2   # jt per exp batch
            nst = CH // 128
            nstg = (CH + 255) // 256  # stage tiles per chunk
            with nc.named_scope(f"att_{tagsfx}"):
                for (i0, _) in _chunks(SQ, CH):
                    ic = i0 // CH
                    for hp in range(H // 2):
                        h0, h1 = 2 * hp, 2 * hp + 1
                        ps_ct0 = psC.tile([128, CH], F32, tag="psC",
                                          name="ps_ct0")
                        ps_ct1 = psC.tile([128, CH], F32, tag="psC",
                                          name="ps_ct1")
                        for jg in range(0, JT, G):
                            assert jg + G <= JT
                            ps_sA = psS.tile([128, G, CH], F32, tag="psSA",
                                             name="ps_sA")
                            ps_sB = psS.tile([128, G, CH], F32, tag="psSB",
                                             name="ps_sB")
                            # score streak; the head pair runs concurrently
                            # on opposite PE row halves / separate banks
                            for g in range(G):
                                jt = jg + g
                                nc.tensor.matmul(
                                    ps_sA[:, g],
                                    ksrc[0:DH, hp, jt * 128:(jt + 1) * 128],
                                    qsrc[0:DH, hp, i0:i0 + CH],
                                    start=True, stop=True)
                                nc.tensor.matmul(
                                    ps_sB[:, g],
                                    ksrc[DH:128, hp, jt * 128:(jt + 1) * 128],
                                    qsrc[DH:128, hp, i0:i0 + CH],
                                    start=True, stop=True)
                            prA = at.tile([128, G, CH], F32R,
                                          tag=f"pr{tagsfx}A", name="prA")
                            prB = at.tile([128, G, CH], F32R,
                                          tag=f"pr{tagsfx}B", name="prB")
                            nc.scalar.activation(prA[:], ps_sA[:], AF.Exp,
                                                 scale=0.125)
                            nc.scalar.activation(prB[:], ps_sB[:], AF.Exp,
                                                 scale=0.125)
                            # ctx streak (K=128, M=65, N=CH)
                            for g in range(G):
                                jt = jg + g
                                nc.tensor.matmul(
                                    ps_ct0[0:DH + 1, :],
                                    vaug[:, jt,
                                         h0 * (DH + 1):(h0 + 1) * (DH + 1)],
                                    prA[:, g],
                                    start=(jt == 0), stop=(jt == JT - 1))
                                nc.tensor.matmul(
                                    ps_ct1[0:DH + 1, :],
                                    vaug[:, jt,
                                         h1 * (DH + 1):(h1 + 1) * (DH + 1)],
                                    prB[:, g],
                                    start=(jt == 0), stop=(jt == JT - 1))
                        # tail: seq-major transpose + normalize
                        for (h, ps_ct) in ((h0, ps_ct0), (h1, ps_ct1)):
                            ctx_sb = at.tile([DH + 1, CH], F32, tag="ctx")
                            nc.vector.tensor_copy(ctx_sb[:], ps_ct[0:DH + 1, :])
                            for ii in range(nst):
                                ps_t = psA.tile([128, 512], F32, tag="psA")
                                nc.tensor.transpose(
                                    ps_t[:, 0:DH + 1],
                                    ctx_sb[:, ii * 128:(ii + 1) * 128],
                                    ident[0:DH + 1, 0:DH + 1])
                                rec = at.tile([128, 1], F32, tag="rec", bufs=4)
                                nc.vector.reciprocal(rec[:], ps_t[:, DH:DH + 1])
                                stage = stages[ic * nstg + ii // 2]
                                nc.vector.tensor_scalar_mul(
                                    stage[:, ii % 2, h * DH:(h + 1) * DH],
                                    ps_t[:, 0:DH], rec[:])
                    # flush this chunk's stage tiles
                    for st2 in range(nstg):
                        st = st_base + (i0 // 128) + 2 * st2
                        stage = stages[ic * nstg + st2]
                        nc.sync.dma_start(cat_dst[:, st:st + 2, :], stage[:])

        def make_stages(at_pool, SQ, sfx):
            return [at_pool.tile([128, 2, D], F32R, tag="stage", bufs=2,
                                 name=f"stage_{sfx}{i}")
                    for i in range((SQ + 255) // 256)]

        def fc_stage(cat_src, wname, bias_bc, dstT, NP, pool):
            wfc_sb = pool.tile([128, ST, NP], F32R, name=f"sb_{wname}")
            nc.gpsimd.dma_start(
                wfc_sb[:], ap[wname].rearrange("(ko p) n -> p ko n", p=128))
            cat_sb = pool.tile([128, ST, D], F32R, name=f"cat_{wname}")
            for st in range(ST):
                nc.sync.dma_start(cat_sb[:, st], cat_src[:, st])
            for mo in range(DT):
                for (n0, nsz) in _chunks(NP, 512):
                    ps = psA.tile([128, 512], F32, tag="psA")
                    for st in range(ST):
                        nc.tensor.matmul(
                            ps[:, 0:nsz],
                            cat_sb[:, st, mo * 128:(mo + 1) * 128],
                            wfc_sb[:, st, n0:n0 + nsz],
                            start=(st == 0), stop=(st == ST - 1))
                    nc.vector.tensor_add(
                        dstT[:, mo, n0:n0 + nsz], ps[:, 0:nsz],
                        bias_bc[:, n0:n0 + nsz])

        def outproj(srcT, wname, bias_bc, out_dram, n_tiles, op, ost):
            wo_sb = op.tile([128, DT, D], F32R, name=f"sb_{wname}")
            nc.gpsimd.dma_start(
                wo_sb[:], ap[wname].rearrange("(ko p) n -> p ko n", p=128))
            for mo in range(n_tiles):
                o_sb = ost.tile([128, D], F32, tag="osb")
                for (n0, nsz) in _chunks(D, 512):
                    ps = psA.tile([128, 512], F32, tag="psA")
                    for kt in range(DT):
                        nc.tensor.matmul(
                            ps[:, 0:nsz],
                            srcT[:, kt, mo * 128:(mo + 1) * 128],
                            wo_sb[:, kt, n0:n0 + nsz],
                            start=(kt == 0), stop=(kt == DT - 1))
                    nc.vector.tensor_add(
                        o_sb[:, n0:n0 + nsz], ps[:, 0:nsz],
                        bias_bc[:, n0:n0 + nsz])
                nc.sync.dma_start(
                    out_dram[mo * 128:(mo + 1) * 128, :], o_sb[:])

        # ---- phase 1: input transposes --------------------------------
        proj_cm = tc.tile_pool(name="proj", bufs=1)
        proj = proj_cm.__enter__()
        molT = proj.tile([128, DT, M], F32R)
        hst_cm = tc.tile_pool(name="hst", bufs=1)
        hstp = hst_cm.__enter__()
        hsT = hstp.tile([128, DT, P], F32R)
        with tc.tile_pool(name="ld", bufs=3) as ld, \
                nc.named_scope("transpose_in"):
            for (src, dstT, nt) in ((ap["hidden_states"], hsT, PT),
                                    (ap["mol"], molT, MT)):
                for st in range(nt):
                    xs = ld.tile([128, D], F32, tag="xs")
                    nc.sync.dma_start(xs[:], src[st * 128:(st + 1) * 128, :])
                    for dt in range(DT):
                        pt = psA.tile([128, 512], F32, tag="psA")
                        nc.tensor.transpose(
                            pt[:, 0:128], xs[:, dt * 128:(dt + 1) * 128],
                            ident[:])
                        nc.vector.tensor_copy(
                            dstT[:, dt, st * 128:(st + 1) * 128],
                            pt[:, 0:128])

        # bias tiles
        bq_p, bk_p = ppart("bq"), ppart("bk")
        bqm_p, bkm_p = ppart("bqm"), ppart("bkm")
        bv_b = bcast("bv", D)
        bvm_b = bcast("bvm", D)
        bfc_b = bcast("bfc", P)
        bfcm_b = bcast("bfc_mol", M)
        bout_b = bcast("bout", D)
        boutm_b = bcast("bout_mol", D)

        # ---- phase 2a: protein projections ----------------------------
        wp1_cm = tc.tile_pool(name="wp1", bufs=2)
        wp_pool[0] = wp1_cm.__enter__()
        with nc.named_scope("proj"):
            project_fm("Wq", bq_p, qT, hsT, P)
            project_fm("Wk", bk_p, kT, hsT, P)
            project_v("Wv", bv_b, v_aug, hsT, PT)
        wp1_cm.__exit__(None, None, None)
        hst_cm.__exit__(None, None, None)

        # ---- pp + pm attention, mol projections as PE filler ----------
        wp2_cm = tc.tile_pool(name="wp2", bufs=2)
        wp_pool[0] = wp2_cm.__enter__()
        atb_cm = tc.tile_pool(name="atb", bufs=2)
        atb = atb_cm.__enter__()
        st_pp = make_stages(atb, P, "pp")
        attention(qT, P, kT, P, v_aug, catp, 0, "pp", atb, st_pp)
        with nc.named_scope("proj_mol"):
            project_fm("Wqm", bqm_p, qmT, molT, M)
            project_fm("Wkm", bkm_p, kmT, molT, M)
            project_v("Wvm", bvm_b, vm_aug, molT, MT)
        st_pm = make_stages(atb, P, "pm")
        attention(qT, P, kmT, M, vm_aug, catm, MT, "pm", atb, st_pm)
        st_mp = make_stages(atb, M, "mp")
        attention(qmT, M, kT, P, v_aug, catp, PT, "mp", atb, st_mp)
        atb_cm.__exit__(None, None, None)
        wp2_cm.__exit__(None, None, None)
        proj_cm.__exit__(None, None, None)
        actsK_cm.__exit__(None, None, None)

        # ---- fc(prot) overlapping mm attention ------------------------
        acts2 = ctx.enter_context(tc.tile_pool(name="acts2", bufs=1))
        ofcT = acts2.tile([128, DT, P], F32R)
        ofcmT = acts2.tile([128, DT, M], F32R)

        fcp_cm = tc.tile_pool(name="fcp", bufs=1)
        fcp = fcp_cm.__enter__()
        with nc.named_scope("fc_prot"):
            fc_stage(catp, "Wfc", bfc_b, ofcT, P, fcp)

        atm_cm = tc.tile_pool(name="atm", bufs=2)
        atm = atm_cm.__enter__()
        st_mm = make_stages(atm, M, "mm")
        attention(qmT, M, kmT, M, vm_aug, catm, 0, "mm", atm, st_mm)
        atm_cm.__exit__(None, None, None)
        fcp_cm.__exit__(None, None, None)

        # ---- outproj(prot) + fc(mol) + outproj(mol) tail --------------
        op = ctx.enter_context(tc.tile_pool(name="op", bufs=1))
        ost = ctx.enter_context(tc.tile_pool(name="ost", bufs=3))
        with nc.named_scope("outproj_prot"):
            outproj(ofcT, "Wout", bout_b, ap["out_prot"], PT, op, ost)

        with tc.tile_pool(name="fcm", bufs=1) as fcm, nc.named_scope("fc_mol"):
            fc_stage(catm, "Wfc_mol", bfcm_b, ofcmT, M, fcm)

        with nc.named_scope("outproj_mol"):
            outproj(ofcmT, "Wout_mol", boutm_b, ap["out_mol"], MT, op, ost)


_NC_CACHE = None


def _get_program():
    global _NC_CACHE
    if _NC_CACHE is None:
        _NC_CACHE = _build()
    return _NC_CACHE


def kernel(**inputs):
    nc = _get_program()
    per_core_names = (["hidden_states", "mol"] + _W_NAMES + _B_NAMES
                      + ["Wfc", "bfc", "Wfc_mol", "bfc_mol"])
    in_maps = []
    for c in range(N_CORES):
        m = {}
        for name in per_core_names:
            arr = np.ascontiguousarray(np.asarray(inputs[name], dtype=np.float32))
            if name in ("hidden_states", "mol"):
                arr = arr[c]
            m[name] = arr
        in_maps.append(m)

    res = bass_utils.run_bass_kernel_spmd(nc, in_maps,
                                          core_ids=list(range(N_CORES)))
    global LAST_RESULTS
    LAST_RESULTS = res
    out_prot = np.stack([res.results[c]["out_prot"] for c in range(N_CORES)])
    out_mol = np.stack([res.results[c]["out_mol"] for c in range(N_CORES)])
    return out_prot, out_mol


LAST_RESULTS = None
